# revision 19
# baseline (speedup 1.0000x reference)
"""Trainium2 Bass kernel (v2) for nn_GRU_43568148251487.

Data-parallel over batch across 8 cores (8 seq/core), 2-layer GRU lag
pipeline (layer 1 lags layer 0 by LAG=32 steps) — one SPMD program, no
collectives. Measured ~8.0 ms vs the 13.6 ms v1 baseline on this target.

Per core, per superstep (one step of each layer):
  - 30 matmuls, N=512 f32r: per layer 12 h@W_hh k-chunk matmuls into three
    per-gate PSUM banks (order r, n, z so the r->sigmoid->n-chain starts
    early) + 3 injects (gi_r, bhn, gi_z) that add the precomputed
    input-gate contributions in PSUM. The injects' stationaries sit at
    disjoint PE row groups (partitions 0/32/64 via inj72) so their
    512-column streams co-execute on different subarrays (~0.6us/superstep
    measured, walrus accepts row positions though not column positions).
  - elementwise pinned per engine: ACT sigmoid(r), sigmoid(z), tanh; DVE
    r*pn, +gi_n, n*(1-z) and hist copies; GPSIMD (1-z), z*h, final add.
    The h'-combine/transpose/hist-copy runs in hidden-halves so the next
    step's k0/k1 matmuls start while half B finishes.
  - 8 PE transposes regenerate hT ([128,8] chunks) into a 16-step history
    buffer that doubles as the stationary operand for the bulk gi1
    (layer-1 input projection) and output-projection matmuls at chunk
    boundaries.

Why this shape: on this part the PE streams moving-operand columns at
~0.85 ns/col (~1.2 GHz, no HAM boost observed) + ~50 ns/matmul overhead,
so the kernel is PE-streaming-bound (~14.6 us/superstep); the vector
engines cost ~1 us/instruction regardless of tile width, so gi injection
stays on the PE (moving it to DVE measured slower) and elementwise op
count is kept minimal. Column tiling (tile_position col!=0) and
fp8-DoubleRow were evaluated and rejected: walrus rejects nonzero dst
partition bases, and fp8 h/W quantization gives ~7e-2 rel err (>2e-2).
f32r keeps rel err ~5.5e-4.
"""
import json
from contextlib import ExitStack

import numpy as np

import concourse.bass as bass
import concourse.tile as tile
from concourse import mybir

f32 = mybir.dt.float32
f32r = mybir.dt.float32r
AO = mybir.AluOpType
AF = mybir.ActivationFunctionType

P = 128
B = 8           # batch per core
NCORES = 8
H = 512
G = 3 * H
KC = 4
CH = 16
LAG = 32
T_FULL = 512

# ---------------------------------------------------------------------------
# Workaround for this walrus build: it rejects >1 sync-wait per instruction.
# Split extra waits onto preceding EventSemaphore instructions on the same
# engine (same-sequencer program order preserves semantics).
_orig_to_json_bytes = bass.Bass.to_json_bytes


def _split_multiwait(mod):
    ctr = [0]

    def mk_es(engine, wait):
        ctr[0] += 1
        return {
            "debug": 0, "engine": engine, "ins": [],
            "name": f"mswsplit-{ctr[0]}", "opcode": "EventSemaphore",
            "outs": [], "sync_info": {"on_update": [], "on_wait": [wait]},
        }

    for fn in mod.get("functions", []):
        for bb in fn.get("blocks", []):
            insts = bb.get("instructions", [])
            if not any(
                len((i.get("sync_info") or {}).get("on_wait") or []) > 1
                for i in insts
            ):
                continue
            out = []
            for inst in insts:
                si = inst.get("sync_info")
                waits = (si or {}).get("on_wait") or []
                if len(waits) > 1:
                    for w in waits[:-1]:
                        out.append(mk_es(inst["engine"], w))
                    si["on_wait"] = [waits[-1]]
                out.append(inst)
            bb["instructions"] = out
    return mod


def _patched_to_json_bytes(self):
    return json.dumps(_split_multiwait(json.loads(_orig_to_json_bytes(self)))).encode()


bass.Bass.to_json_bytes = _patched_to_json_bytes


# ---------------------------------------------------------------------------
def _rnz(a):
    # reorder gate columns [r|z|n] -> [r|n|z] (last axis, size G)
    return np.ascontiguousarray(
        np.concatenate([a[..., 0:H], a[..., 2 * H:3 * H], a[..., H:2 * H]], -1))


def _inj72():
    m = np.zeros((72, B), np.float32)
    m[0:B, :] = np.eye(B)
    m[32:32 + B, :] = np.eye(B)
    m[64, :] = 1.0
    return m


def _host_prep(inputs, core):
    x = np.ascontiguousarray(np.asarray(inputs["x"], np.float32)[core * B:(core + 1) * B])
    t = np.ascontiguousarray(np.asarray(inputs["t"], np.float32)[core * B:(core + 1) * B])
    T = x.shape[1]
    g = {k: np.asarray(v, np.float32) for k, v in inputs.items()}

    def kchunked(WT):
        F = WT.shape[1]
        return np.ascontiguousarray(
            WT.reshape(KC, P, F).transpose(1, 0, 2).reshape(P, KC * F)
        ).astype(np.float32)

    b0 = g["b_ih0"] + g["W_ih0"] @ (g["bx"] + g["bt"])
    b0 = b0.copy()
    b0[:2 * H] += g["b_hh0"][:2 * H]
    b1 = g["b_ih1"].copy()
    b1[:2 * H] += g["b_hh1"][:2 * H]
    return {
        "xT": np.ascontiguousarray(x.reshape(B * T, 64).T),
        "tT": np.ascontiguousarray(t.reshape(B * T, 1).T),
        "giA": _rnz((g["W_ih0"] @ g["Wx"]).T),
        "giB": _rnz((g["W_ih0"] @ g["Wt"]).T),
        "whhT0": kchunked(g["W_hh0"].T), "whhT1": kchunked(g["W_hh1"].T),
        "wihT1": kchunked(_rnz(g["W_ih1"].T)),
        "bias0": np.ascontiguousarray(np.broadcast_to(_rnz(b0[None, :])[0], (P, G))).astype(np.float32),
        "bias1": np.ascontiguousarray(np.broadcast_to(_rnz(b1[None, :])[0], (P, G))).astype(np.float32),
        "bhn0": np.ascontiguousarray(g["b_hh0"][2 * H:][None, :]),
        "bhn1": np.ascontiguousarray(g["b_hh1"][2 * H:][None, :]),
        "ones8": np.ones((1, B), np.float32),
        "inj72": _inj72(),
        "id8": np.eye(B, dtype=np.float32),
        "id8r": np.eye(B, dtype=np.float32),
        "woT": kchunked(g["Wo"].T),
        "z64": np.zeros((P, 2 * KC * B), np.float32),
        "bo_bc": np.ascontiguousarray(np.broadcast_to(g["bo"], (P, 64))).astype(np.float32),
    }


def _build(T):
    assert T % CH == 0
    NCHUNK = T // CH
    NSS = T + LAG
    ROWS = B * T

    nc = bass.Bass("TRN2", debug=False, num_devices=NCORES)

    d = {}
    d["xT"] = nc.dram_tensor("xT", [64, ROWS], f32r, kind="ExternalInput")
    d["tT"] = nc.dram_tensor("tT", [1, ROWS], f32r, kind="ExternalInput")
    d["giA"] = nc.dram_tensor("giA", [64, G], f32r, kind="ExternalInput")
    d["giB"] = nc.dram_tensor("giB", [1, G], f32r, kind="ExternalInput")
    d["whhT0"] = nc.dram_tensor("whhT0", [P, KC * G], f32r, kind="ExternalInput")
    d["whhT1"] = nc.dram_tensor("whhT1", [P, KC * G], f32r, kind="ExternalInput")
    d["wihT1"] = nc.dram_tensor("wihT1", [P, KC * G], f32r, kind="ExternalInput")
    d["bias0"] = nc.dram_tensor("bias0", [P, G], f32, kind="ExternalInput")
    d["bias1"] = nc.dram_tensor("bias1", [P, G], f32, kind="ExternalInput")
    d["bhn0"] = nc.dram_tensor("bhn0", [1, H], f32r, kind="ExternalInput")
    d["bhn1"] = nc.dram_tensor("bhn1", [1, H], f32r, kind="ExternalInput")
    d["ones8"] = nc.dram_tensor("ones8", [1, B], f32r, kind="ExternalInput")
    d["inj72"] = nc.dram_tensor("inj72", [72, B], f32r, kind="ExternalInput")
    d["id8"] = nc.dram_tensor("id8", [B, B], f32, kind="ExternalInput")
    d["id8r"] = nc.dram_tensor("id8r", [B, B], f32r, kind="ExternalInput")
    d["woT"] = nc.dram_tensor("woT", [P, KC * 64], f32r, kind="ExternalInput")
    d["z64"] = nc.dram_tensor("z64", [P, 2 * KC * B], f32r, kind="ExternalInput")
    d["bo_bc"] = nc.dram_tensor("bo_bc", [P, 64], f32, kind="ExternalInput")
    out_d = nc.dram_tensor("out", [B, T, 64], f32, kind="ExternalOutput")

    with tile.TileContext(nc) as tc, ExitStack() as ctx:
        wp = ctx.enter_context(tc.tile_pool(name="wp", bufs=1))
        dramp = ctx.enter_context(tc.tile_pool(name="dramp", bufs=1, space="DRAM"))

        def load(name, shape, dt):
            tl = wp.tile(shape, dt, name=f"w_{name}")
            nc.sync.dma_start(tl[:], d[name].ap())
            return tl

        whhT = [load("whhT0", [P, KC * G], f32r), load("whhT1", [P, KC * G], f32r)]
        wihT1 = load("wihT1", [P, KC * G], f32r)
        bias0 = load("bias0", [P, G], f32)
        bias1 = load("bias1", [P, G], f32)
        bhn = [load("bhn0", [1, H], f32r), load("bhn1", [1, H], f32r)]
        ones8 = load("ones8", [1, B], f32r)
        inj72 = load("inj72", [72, B], f32r)
        bhn64 = [wp.tile([65, H], f32r, name=f"bhn64_{l}") for l in range(2)]
        for l in range(2):
            nc.sync.dma_start(bhn64[l][64:65, :], d[f"bhn{l}"].ap())
        id8 = load("id8", [B, B], f32)
        id8r = load("id8r", [B, B], f32r)
        woT = load("woT", [P, KC * 64], f32r)
        bo_bc = load("bo_bc", [P, 64], f32)
        giA = load("giA", [64, G], f32r)
        giB = load("giB", [1, G], f32r)

        hist_init = wp.tile([P, 2, KC, B], f32r, name="hist_init")
        nc.sync.dma_start(hist_init[:].rearrange("p a b c -> p (a b c)"), d["z64"].ap())
        hb_init = [wp.tile([B, H], f32, name=f"hb_init{l}") for l in range(2)]
        for tl in hb_init:
            nc.vector.memset(tl[:], 0.0)

        gi_d = [
            dramp.tile([B, T, G], f32r, name="gi0_d"),
            dramp.tile([B, T, G], f32r, name="gi1_d"),
        ]
        gi0_rows = gi_d[0][:].rearrange("b t f -> (b t) f")

        # Phase A: gi0 = x @ (W_ih0 Wx).T + t @ (W_ih0 Wt).T + bias0
        with tc.tile_pool(name="pA", bufs=3) as pA, \
             tc.tile_pool(name="pAx", bufs=1) as pAx, \
             tc.tile_pool(name="psA", bufs=2, space="PSUM") as psA:
            xT_sb = pAx.tile([64, ROWS], f32r, name="xT_sb")
            nc.sync.dma_start(xT_sb[:], d["xT"].ap())
            tT_sb = pAx.tile([1, ROWS], f32r, name="tT_sb")
            nc.sync.dma_start(tT_sb[:], d["tT"].ap())
            for mt in range(ROWS // P):
                gi_sb = pA.tile([P, G], f32r, name="gi0_sb")
                for gg in range(3):
                    acc = psA.tile([P, 512], f32, name="accA")
                    nc.tensor.matmul(acc[:], xT_sb[:, mt * P:(mt + 1) * P],
                                     giA[:, gg * 512:(gg + 1) * 512],
                                     start=True, stop=False)
                    nc.tensor.matmul(acc[:], tT_sb[:, mt * P:(mt + 1) * P],
                                     giB[:, gg * 512:(gg + 1) * 512],
                                     start=False, stop=True)
                    nc.vector.tensor_tensor(
                        gi_sb[:, gg * 512:(gg + 1) * 512], acc[:],
                        bias0[:, gg * 512:(gg + 1) * 512], AO.add)
                nc.sync.dma_start(gi0_rows[mt * P:(mt + 1) * P, :], gi_sb[:])

        # Phase B: recurrence.  Per layer-step: 3 PSUM banks (r, n, z), each
        # filled by 4 accumulating k-matmuls + 1 inject, order r, n, z.
        with tc.tile_pool(name="pg", bufs=4) as pg, \
             tc.tile_pool(name="ph", bufs=2) as ph, \
             tc.tile_pool(name="pt", bufs=1) as pt, \
             tc.tile_pool(name="pth", bufs=2) as pth, \
             tc.tile_pool(name="pb", bufs=2) as pb, \
             tc.tile_pool(name="psG", bufs=1, space="PSUM") as psG, \
             tc.tile_pool(name="psT", bufs=1, space="PSUM") as psT, \
             tc.tile_pool(name="psB", bufs=1, space="PSUM") as psB:

            hb_prev = [hb_init[0], hb_init[1]]
            hT_prev = [[hist_init[:, l, k, :] for k in range(KC)] for l in range(2)]
            hist_cur = None

            for s in range(NSS):
                act = [s < T, s >= LAG]
                t1 = s - LAG
                sidx = s % CH
                if sidx == 0:
                    hist_cur = ph.tile([P, 2, KC, B, CH], f32r, name="hist")

                gis = [None, None]
                for l, tt in ((0, s), (1, t1)):
                    if not act[l]:
                        continue
                    gt = pg.tile([72, 1024], f32r, name=f"gi{l}_t", tag=f"gi{l}_t")
                    nc.sync.dma_start(gt[0:8, 0:1024], gi_d[l][:, tt, 0:1024])
                    nc.sync.dma_start(gt[32:40, 0:512], gi_d[l][:, tt, 1024:1536])
                    gis[l] = gt

                # --- matmuls: r-block, n-block, z-block per active layer ---
                pr = [None, None]
                pz = [None, None]
                pn = [None, None]
                for l in range(2):
                    if not act[l]:
                        continue
                    w = whhT[l]
                    pr[l] = psG.tile([B, 512], f32, name=f"pr{l}")
                    pn[l] = psG.tile([B, 512], f32, name=f"pn{l}")
                    pz[l] = psG.tile([B, 512], f32, name=f"pz{l}")
                    # three injects adjacent at row groups 0/64/32 so their
                    # 512-col streams co-execute on disjoint PE subarrays
                    nc.tensor.matmul(pr[l][:], inj72[0:8, :], gis[l][0:8, 0:512],
                                     start=True, stop=False, skip_group_check=True)
                    nc.tensor.matmul(pn[l][:], inj72[64:65, :], bhn64[l][64:65, :],
                                     start=True, stop=False, skip_group_check=True)
                    nc.tensor.matmul(pz[l][:], inj72[32:40, :], gis[l][32:40, 0:512],
                                     start=True, stop=False, skip_group_check=True)
                    # k-matmuls: r block, n block, z block
                    for k in range(KC):
                        nc.tensor.matmul(pr[l][:], hT_prev[l][k],
                                         w[:, k * G + 0:k * G + 512],
                                         start=False, stop=(k == KC - 1),
                                         skip_group_check=True)
                    for k in range(KC):
                        nc.tensor.matmul(pn[l][:], hT_prev[l][k],
                                         w[:, k * G + 1024:k * G + 1536],
                                         start=False, stop=(k == KC - 1),
                                         skip_group_check=True)
                    for k in range(KC):
                        nc.tensor.matmul(pz[l][:], hT_prev[l][k],
                                         w[:, k * G + 512:k * G + 1024],
                                         start=False, stop=(k == KC - 1),
                                         skip_group_check=True)

                # --- elementwise ---
                hb_new = [None, None]
                phT = psT.tile([P, 2, KC, B], f32, name="phT")
                for l in range(2):
                    if not act[l]:
                        continue
                    g = gis[l]
                    r = pt.tile([B, H], f32, name=f"r{l}")
                    nc.scalar.activation(r[:], pr[l][:], AF.Sigmoid)
                    mn = pt.tile([B, H], f32, name=f"mn{l}")
                    nc.vector.tensor_tensor(mn[:], pn[l][:], r[:], AO.mult)
                    nc.vector.tensor_tensor(mn[:], mn[:], g[0:8, 512:1024], AO.add)
                    z = pt.tile([B, H], f32, name=f"z{l}")
                    nc.scalar.activation(z[:], pz[l][:], AF.Sigmoid)
                    n = pt.tile([B, H], f32, name=f"n{l}")
                    nc.scalar.activation(n[:], mn[:], AF.Tanh)
                    tz = pt.tile([B, H], f32, name=f"tz{l}")
                    nc.gpsimd.tensor_scalar(tz[:], z[:], -1.0, 1.0, AO.mult, AO.add)
                    w_ = pt.tile([B, H], f32, name=f"w{l}")
                    nc.gpsimd.tensor_tensor(w_[:], z[:], hb_prev[l][:], AO.mult)
                    # combine + transpose + hist copy in hidden-halves, so the
                    # next step's k0/k1 matmuls start while half B finishes.
                    hb = pth.tile([B, H], f32, name=f"hb{l}")
                    HH = H // 2
                    for hf in range(2):
                        sl = slice(hf * HH, (hf + 1) * HH)
                        nc.vector.tensor_tensor(hb[:, sl], n[:, sl], tz[:, sl], AO.mult)
                        nc.vector.tensor_tensor(hb[:, sl], hb[:, sl], w_[:, sl], AO.add)
                        for k in (2 * hf, 2 * hf + 1):
                            nc.tensor.transpose(phT[:, l, k, :],
                                                hb[:, k * P:(k + 1) * P], id8[:])
                        nc.vector.tensor_copy(
                            hist_cur[:, l, 2 * hf:2 * hf + 2, :, sidx],
                            phT[:, l, 2 * hf:2 * hf + 2, :])
                    hb_new[l] = hb

                for l in range(2):
                    if act[l]:
                        hb_prev[l] = hb_new[l]
                        hT_prev[l] = [hist_cur[:, l, k, :, sidx] for k in range(KC)]

                # --- chunk-boundary bulk work ---
                if sidx == CH - 1:
                    c = s // CH
                    if c < NCHUNK:
                        gi1_sb = pb.tile([P, G], f32r, name="gi1_sb")
                        for gg in range(3):
                            accb = psB.tile([P, 512], f32, name="accB", tag="accB")
                            for k in range(KC):
                                nc.tensor.matmul(
                                    accb[:], hist_cur[:, 0, k, :, :],
                                    wihT1[:, k * G + gg * 512:k * G + (gg + 1) * 512],
                                    start=(k == 0), stop=(k == KC - 1))
                            nc.vector.tensor_tensor(
                                gi1_sb[:, gg * 512:(gg + 1) * 512], accb[:],
                                bias1[:, gg * 512:(gg + 1) * 512], AO.add)
                        for b_ in range(B):
                            nc.sync.dma_start(
                                gi_d[1][b_, c * CH:(c + 1) * CH, :],
                                gi1_sb[b_ * CH:(b_ + 1) * CH, :])
                    if s >= LAG + CH - 1:
                        t0 = c * CH - LAG
                        rel = pb.tile([P, KC, B, CH], f32r, name="relu_sb")
                        nc.scalar.activation(rel[:], hist_cur[:, 1, :, :, :], AF.Relu)
                        acco_full = psB.tile([P, 512], f32, name="accO", tag="accB")
                        acco = acco_full[:, 0:64]
                        for k in range(KC):
                            nc.tensor.matmul(acco[:, :], rel[:, k, :, :],
                                             woT[:, k * 64:(k + 1) * 64],
                                             start=(k == 0), stop=(k == KC - 1))
                        out_sb = pb.tile([P, 64], f32, name="out_sb")
                        nc.vector.tensor_tensor(out_sb[:], acco[:], bo_bc[:], AO.add)
                        for b_ in range(B):
                            nc.sync.dma_start(
                                out_d.ap()[b_, t0:t0 + CH, :],
                                out_sb[b_ * CH:(b_ + 1) * CH, :])
    return nc


_NC_CACHE = {}


def _get_nc(T):
    if T not in _NC_CACHE:
        _NC_CACHE[T] = _build(T)
    return _NC_CACHE[T]


def kernel(**inputs):
    from concourse.bass_utils import run_bass_kernel_spmd

    T = np.asarray(inputs["x"]).shape[1]
    nc = _get_nc(T)
    in_maps = [_host_prep(inputs, c) for c in range(NCORES)]
    res = run_bass_kernel_spmd(nc, in_maps, core_ids=list(range(NCORES)))
    out = np.concatenate([res.results[c]["out"] for c in range(NCORES)], axis=0)
    return out.astype(np.float32)
